# revision 15
# baseline (speedup 1.0000x reference)
"""CorrectedLinear on 8 TRN2 NeuronCores.

Math: out = x @ W.T + b + (x @ V_r) @ C.T
    = x @ (W.T + V_r @ C.T) + b          -- fold the rank-32 correction
      into a single effective weight matrix Wt [d_in, d_out] (0.05% of
      the GEMM FLOPs, done host-side in float64).

Sharding: pure data-parallel over the batch dim (8 batches -> 8 cores).
Each core computes a [8192, 1024] x [1024, 1024] GEMM.

Device layout: the PE contracts along the partition axis, so both
operands need d_in on partitions. x is fed pre-transposed per core
(xT [d_in, t]) and the output is produced transposed (outT [d_out, t]),
un-transposed on the host. All matmul operands use float32r (full-rate
fp32, ~1.5e-4 matmul precision) with fp32 PSUM accumulation.

Weights are fed as one 512 KB contiguous SBUF-image per 128-row output
slice (4 KB per partition per DMA) on the ACT HWDGE ring, x streams on
the SP ring, and the first/last chunks are halved so matmul groups are
gated on 1 MB rather than 2 MB arrivals.
"""

import numpy as np

N_CORES = 8
T = 8192          # tokens per core (batch entry)
D = 1024          # d_in
O = 1024          # d_out
TCH = 512         # moving free dim per matmul (= one PSUM bank of fp32)
NT = T // TCH     # 16 t-chunks
ND = D // 128     # 8 contraction slices
NO = O // 128     # 8 output-partition slices

# chunks processed as two 256-token halves (finer DMA-arrival pacing at
# the start; smaller final evac+store at the end)
_HALVED = ()

_nc = None


def _build():
    import concourse.bacc as bacc
    import concourse.mybir as mybir
    import concourse.tile as tile

    f32 = mybir.dt.float32
    f32r = mybir.dt.float32r

    nc = bacc.Bacc(
        "TRN2", target_bir_lowering=False, debug=False, num_devices=N_CORES
    )
    xT_d = nc.dram_tensor("xT", [D, T], f32r, kind="ExternalInput")
    Wb_d = nc.dram_tensor("Wb", [NO, 128, ND * 128], f32r, kind="ExternalInput")
    b_d = nc.dram_tensor("bb", [O], f32, kind="ExternalInput")
    outT_d = nc.dram_tensor("outT", [O, T], f32, kind="ExternalOutput")

    with tile.TileContext(nc) as tc:
        with (
            tc.tile_pool(name="wt", bufs=1) as wt_pool,
            tc.tile_pool(name="bp", bufs=1) as b_pool,
            tc.tile_pool(name="xp", bufs=48) as x_pool,
            tc.tile_pool(name="op", bufs=8) as o_pool,
            tc.tile_pool(name="ps", bufs=8, space="PSUM") as psum_pool,
        ):
            b_sb = b_pool.tile([128, NO], f32, name="b_sb")
            nc.scalar.dma_start(
                out=b_sb[:], in_=b_d.ap().rearrange("(j p) -> p j", p=128)
            )

            def load_w(o):
                w = wt_pool.tile([128, ND * 128], f32r, name=f"w{o}", tag=f"w{o}")
                nc.scalar.dma_start(out=w[:], in_=Wb_d.ap()[o])
                return w

            def evac_and_store(o, acc, t0, tn, parallel=False):
                """PSUM -> SBUF (+bias) -> DRAM, alternating ACT/DVE.

                parallel=True splits the tile in half across both engines
                and both HWDGE rings to shorten the kernel tail."""
                if parallel:
                    hn = tn // 2
                    ot = o_pool.tile([128, tn], f32, name="ot", tag="ot")
                    nc.scalar.activation(
                        ot[:, :hn],
                        acc[:, :hn],
                        mybir.ActivationFunctionType.Identity,
                        bias=b_sb[:, o : o + 1],
                    )
                    nc.vector.tensor_scalar_add(
                        ot[:, hn:], acc[:, hn:], b_sb[:, o : o + 1]
                    )
                    nc.scalar.dma_start(
                        out=outT_d.ap()[o * 128 : (o + 1) * 128, t0 : t0 + hn],
                        in_=ot[:, :hn],
                    )
                    nc.sync.dma_start(
                        out=outT_d.ap()[o * 128 : (o + 1) * 128, t0 + hn : t0 + tn],
                        in_=ot[:, hn:],
                    )
                    return
                ot = o_pool.tile([128, tn], f32, name="ot", tag="ot")
                if o % 2 == 0:
                    nc.scalar.activation(
                        ot[:],
                        acc[:],
                        mybir.ActivationFunctionType.Identity,
                        bias=b_sb[:, o : o + 1],
                    )
                else:
                    nc.vector.tensor_scalar_add(ot[:], acc[:], b_sb[:, o : o + 1])
                nc.scalar.dma_start(
                    out=outT_d.ap()[o * 128 : (o + 1) * 128, t0 : t0 + tn],
                    in_=ot[:],
                )

            def load_x(t0, tn):
                xs = []
                for d in range(ND):
                    xt = x_pool.tile([128, tn], f32r, name="xt", tag="xt")
                    nc.sync.dma_start(
                        out=xt[:],
                        in_=xT_d.ap()[d * 128 : (d + 1) * 128, t0 : t0 + tn],
                    )
                    xs.append(xt)
                return xs

            def mm_groups(x_tiles, t0, tn, last=False):
                for o in range(NO):
                    acc = psum_pool.tile([128, tn], f32, name="acc", tag="acc")
                    for d in range(ND):
                        nc.tensor.matmul(
                            acc[:],
                            wts[o][d],
                            x_tiles[d][:],
                            start=(d == 0),
                            stop=(d == ND - 1),
                        )
                    evac_and_store(o, acc, t0, tn, parallel=last)

            # DMA issue order: b, w0, chunk-0 x, then w1..w7. The HWDGE
            # completion sems are 8 round-robin lanes, so any DMA issued
            # before chunk-0's x tiles becomes a false dependency of the
            # first matmul group — keep exactly b + w0 ahead of it.
            w_os = [load_w(0)]
            x_chunks = [load_x(0, TCH)]
            for o in range(1, NO):
                w_os.append(load_w(o))
            wts = [
                [w_os[o][:, d * 128 : (d + 1) * 128] for d in range(ND)]
                for o in range(NO)
            ]
            for t in range(NT):
                if t + 1 < NT:
                    x_chunks.append(load_x((t + 1) * TCH, TCH))
                mm_groups(x_chunks[t], t * TCH, TCH, last=(t == NT - 1))
    nc.compile()
    return nc


def _get_nc():
    global _nc
    if _nc is None:
        _nc = _build()
    return _nc


def _make_in_maps(x, W, b, V_r, C):
    Wt = (
        W.astype(np.float64).T + V_r.astype(np.float64) @ C.astype(np.float64).T
    ).astype(np.float32)
    # per-o SBUF image: Wb[o, p, 128d+c] = Wt[128d+p, 128o+c] so each
    # o-slice is one 512 KB DMA with 4 KB contiguous per partition
    Wb = np.ascontiguousarray(
        Wt.reshape(ND, 128, NO, 128).transpose(2, 1, 0, 3).reshape(NO, 128, ND * 128)
    )
    b = np.ascontiguousarray(b, dtype=np.float32)
    return [
        {
            "xT": np.ascontiguousarray(x[i].T.astype(np.float32, copy=False)),
            "Wb": Wb,
            "bb": b,
        }
        for i in range(N_CORES)
    ]


def _execute(in_maps, trace=False):
    from concourse.bass_utils import run_bass_kernel_spmd

    return run_bass_kernel_spmd(
        _get_nc(), in_maps, list(range(N_CORES)), trace=trace
    )


def kernel(x, W, b, V_r, C):
    res = _execute(_make_in_maps(x, W, b, V_r, C))
    out = np.empty((N_CORES, T, O), dtype=np.float32)
    for i in range(N_CORES):
        out[i] = res.results[i]["outT"].T
    return out


# revision 16
# speedup vs baseline: 1.0157x; 1.0157x over previous
"""CorrectedLinear on 8 TRN2 NeuronCores.

Math: out = x @ W.T + b + (x @ V_r) @ C.T
    = x @ (W.T + V_r @ C.T) + b          -- fold the rank-32 correction
      into a single effective weight matrix Wt [d_in, d_out] (0.05% of
      the GEMM FLOPs, done host-side in float64).

Sharding: pure data-parallel over the batch dim (8 batches -> 8 cores).
Each core computes a [8192, 1024] x [1024, 1024] GEMM.

Device layout: the PE contracts along the partition axis, so both
operands need d_in on partitions. x is fed pre-transposed per core
(xT [d_in, t]) and the output is produced transposed (outT [d_out, t]),
un-transposed on the host. All matmul operands use float32r (full-rate
fp32, ~1.5e-4 matmul precision) with fp32 PSUM accumulation.

Weights are fed as one 512 KB contiguous SBUF-image per 128-row output
slice (4 KB per partition per DMA) on the ACT HWDGE ring, x streams on
the SP ring, and the first/last chunks are halved so matmul groups are
gated on 1 MB rather than 2 MB arrivals.
"""

import numpy as np

N_CORES = 8
T = 8192          # tokens per core (batch entry)
D = 1024          # d_in
O = 1024          # d_out
TCH = 512         # moving free dim per matmul (= one PSUM bank of fp32)
NT = T // TCH     # 16 t-chunks
ND = D // 128     # 8 contraction slices
NO = O // 128     # 8 output-partition slices

# chunks processed as two 256-token halves (finer DMA-arrival pacing at
# the start; smaller final evac+store at the end)
_HALVED = ()

_nc = None


def _build():
    import concourse.bacc as bacc
    import concourse.mybir as mybir
    import concourse.tile as tile

    f32 = mybir.dt.float32
    f32r = mybir.dt.float32r

    nc = bacc.Bacc(
        "TRN2", target_bir_lowering=False, debug=False, num_devices=N_CORES
    )
    xT_d = nc.dram_tensor("xT", [D, T], f32r, kind="ExternalInput")
    Wb_d = nc.dram_tensor("Wb", [NO, 128, ND * 128], f32r, kind="ExternalInput")
    b_d = nc.dram_tensor("bb", [O], f32, kind="ExternalInput")
    outT_d = nc.dram_tensor("outT", [O, T], f32, kind="ExternalOutput")

    with tile.TileContext(nc) as tc:
        with (
            tc.tile_pool(name="wt", bufs=1) as wt_pool,
            tc.tile_pool(name="bp", bufs=1) as b_pool,
            tc.tile_pool(name="xp", bufs=48) as x_pool,
            tc.tile_pool(name="op", bufs=8) as o_pool,
            tc.tile_pool(name="ps", bufs=8, space="PSUM") as psum_pool,
        ):
            b_sb = b_pool.tile([128, NO], f32, name="b_sb")
            nc.scalar.dma_start(
                out=b_sb[:], in_=b_d.ap().rearrange("(j p) -> p j", p=128)
            )

            def load_w(o):
                w = wt_pool.tile([128, ND * 128], f32r, name=f"w{o}", tag=f"w{o}")
                nc.scalar.dma_start(out=w[:], in_=Wb_d.ap()[o])
                return w

            def evac_and_store(o, acc, t0, tn, parallel=False):
                """PSUM -> SBUF (+bias) -> DRAM, alternating ACT/DVE.

                parallel=True splits the tile in half across both engines
                and both HWDGE rings to shorten the kernel tail."""
                if parallel:
                    hn = tn // 2
                    ot = o_pool.tile([128, tn], f32, name="ot", tag="ot")
                    nc.scalar.activation(
                        ot[:, :hn],
                        acc[:, :hn],
                        mybir.ActivationFunctionType.Identity,
                        bias=b_sb[:, o : o + 1],
                    )
                    nc.vector.tensor_scalar_add(
                        ot[:, hn:], acc[:, hn:], b_sb[:, o : o + 1]
                    )
                    nc.scalar.dma_start(
                        out=outT_d.ap()[o * 128 : (o + 1) * 128, t0 : t0 + hn],
                        in_=ot[:, :hn],
                    )
                    nc.sync.dma_start(
                        out=outT_d.ap()[o * 128 : (o + 1) * 128, t0 + hn : t0 + tn],
                        in_=ot[:, hn:],
                    )
                    return
                ot = o_pool.tile([128, tn], f32, name="ot", tag="ot")
                if o % 2 == 0:
                    nc.scalar.activation(
                        ot[:],
                        acc[:],
                        mybir.ActivationFunctionType.Identity,
                        bias=b_sb[:, o : o + 1],
                    )
                else:
                    nc.vector.tensor_scalar_add(ot[:], acc[:], b_sb[:, o : o + 1])
                nc.scalar.dma_start(
                    out=outT_d.ap()[o * 128 : (o + 1) * 128, t0 : t0 + tn],
                    in_=ot[:],
                )

            def load_x(t0, tn):
                xs = []
                for d in range(ND):
                    xt = x_pool.tile([128, tn], f32r, name="xt", tag="xt")
                    nc.sync.dma_start(
                        out=xt[:],
                        in_=xT_d.ap()[d * 128 : (d + 1) * 128, t0 : t0 + tn],
                    )
                    xs.append(xt)
                return xs

            def mm_groups(x_tiles, t0, tn, last=False):
                for o in range(NO):
                    acc = psum_pool.tile([128, tn], f32, name="acc", tag="acc")
                    for d in range(ND):
                        nc.tensor.matmul(
                            acc[:],
                            wts[o][d],
                            x_tiles[d][:],
                            start=(d == 0),
                            stop=(d == ND - 1),
                        )
                    evac_and_store(o, acc, t0, tn, parallel=last)

            # DMA issue order: b + all weights on the ACT ring first (4 MB,
            # fully landed by ~12us), x chunks on the SP ring — the first
            # matmul group waits ~15us for chunk-0 x, but after that the PE
            # is never weight-gated (loading weights after chunk-0 x instead
            # measured worse: it trades head position for pacing stalls).
            w_os = [load_w(o) for o in range(NO)]
            x_chunks = [load_x(0, TCH)]
            wts = [
                [w_os[o][:, d * 128 : (d + 1) * 128] for d in range(ND)]
                for o in range(NO)
            ]
            for t in range(NT):
                if t + 1 < NT:
                    x_chunks.append(load_x((t + 1) * TCH, TCH))
                mm_groups(x_chunks[t], t * TCH, TCH, last=(t == NT - 1))
    nc.compile()
    return nc


def _get_nc():
    global _nc
    if _nc is None:
        _nc = _build()
    return _nc


def _make_in_maps(x, W, b, V_r, C):
    Wt = (
        W.astype(np.float64).T + V_r.astype(np.float64) @ C.astype(np.float64).T
    ).astype(np.float32)
    # per-o SBUF image: Wb[o, p, 128d+c] = Wt[128d+p, 128o+c] so each
    # o-slice is one 512 KB DMA with 4 KB contiguous per partition
    Wb = np.ascontiguousarray(
        Wt.reshape(ND, 128, NO, 128).transpose(2, 1, 0, 3).reshape(NO, 128, ND * 128)
    )
    b = np.ascontiguousarray(b, dtype=np.float32)
    return [
        {
            "xT": np.ascontiguousarray(x[i].T.astype(np.float32, copy=False)),
            "Wb": Wb,
            "bb": b,
        }
        for i in range(N_CORES)
    ]


def _execute(in_maps, trace=False):
    from concourse.bass_utils import run_bass_kernel_spmd

    return run_bass_kernel_spmd(
        _get_nc(), in_maps, list(range(N_CORES)), trace=trace
    )


def kernel(x, W, b, V_r, C):
    res = _execute(_make_in_maps(x, W, b, V_r, C))
    out = np.empty((N_CORES, T, O), dtype=np.float32)
    for i in range(N_CORES):
        out[i] = res.results[i]["outT"].T
    return out


# revision 17
# speedup vs baseline: 1.0163x; 1.0006x over previous
"""CorrectedLinear on 8 TRN2 NeuronCores.

Math: out = x @ W.T + b + (x @ V_r) @ C.T
    = x @ (W.T + V_r @ C.T) + b          -- fold the rank-32 correction
      into a single effective weight matrix Wt [d_in, d_out] (0.05% of
      the GEMM FLOPs, done host-side in float64).

Sharding: pure data-parallel over the batch dim (8 batches -> 8 cores).
Each core computes a [8192, 1024] x [1024, 1024] GEMM.

Device layout: the PE contracts along the partition axis, so both
operands need d_in on partitions. x is fed pre-transposed per core
(xT [d_in, t]) and the output is produced transposed (outT [d_out, t]),
un-transposed on the host. All matmul operands use float32r (full-rate
fp32, ~1.5e-4 matmul precision) with fp32 PSUM accumulation.

Weights are fed as one 512 KB contiguous SBUF-image per 128-row output
slice (4 KB per partition per DMA) on the ACT HWDGE ring, x streams on
the SP ring, and the first/last chunks are halved so matmul groups are
gated on 1 MB rather than 2 MB arrivals.
"""

import numpy as np

N_CORES = 8
T = 8192          # tokens per core (batch entry)
D = 1024          # d_in
O = 1024          # d_out
TCH = 512         # moving free dim per matmul (= one PSUM bank of fp32)
NT = T // TCH     # 16 t-chunks
ND = D // 128     # 8 contraction slices
NO = O // 128     # 8 output-partition slices

# chunks processed as two 256-token halves (finer DMA-arrival pacing at
# the start; smaller final evac+store at the end)
_HALVED = ()

_nc = None


def _build():
    import concourse.bacc as bacc
    import concourse.mybir as mybir
    import concourse.tile as tile

    f32 = mybir.dt.float32
    f32r = mybir.dt.float32r

    nc = bacc.Bacc(
        "TRN2", target_bir_lowering=False, debug=False, num_devices=N_CORES
    )
    xT_d = nc.dram_tensor("xT", [D, T], f32r, kind="ExternalInput")
    Wb_d = nc.dram_tensor("Wb", [NO, 128, ND * 128], f32r, kind="ExternalInput")
    b_d = nc.dram_tensor("bb", [O], f32, kind="ExternalInput")
    outT_d = nc.dram_tensor("outT", [O, T], f32, kind="ExternalOutput")

    with tile.TileContext(nc) as tc:
        with (
            tc.tile_pool(name="wt", bufs=1) as wt_pool,
            tc.tile_pool(name="bp", bufs=1) as b_pool,
            tc.tile_pool(name="xp", bufs=6) as x_pool,
            tc.tile_pool(name="op", bufs=8) as o_pool,
            tc.tile_pool(name="ps", bufs=8, space="PSUM") as psum_pool,
        ):
            b_sb = b_pool.tile([128, NO], f32, name="b_sb")
            nc.scalar.dma_start(
                out=b_sb[:], in_=b_d.ap().rearrange("(j p) -> p j", p=128)
            )

            def load_w(o):
                w = wt_pool.tile([128, ND * 128], f32r, name=f"w{o}", tag=f"w{o}")
                nc.scalar.dma_start(out=w[:], in_=Wb_d.ap()[o])
                return w

            def evac_and_store(o, acc, t0, tn, parallel=False):
                """PSUM -> SBUF (+bias) -> DRAM, alternating ACT/DVE.

                parallel=True splits the tile in half across both engines
                and both HWDGE rings to shorten the kernel tail."""
                if parallel:
                    hn = tn // 2
                    ot = o_pool.tile([128, tn], f32, name="ot", tag="ot")
                    nc.scalar.activation(
                        ot[:, :hn],
                        acc[:, :hn],
                        mybir.ActivationFunctionType.Identity,
                        bias=b_sb[:, o : o + 1],
                    )
                    nc.vector.tensor_scalar_add(
                        ot[:, hn:], acc[:, hn:], b_sb[:, o : o + 1]
                    )
                    nc.scalar.dma_start(
                        out=outT_d.ap()[o * 128 : (o + 1) * 128, t0 : t0 + hn],
                        in_=ot[:, :hn],
                    )
                    nc.sync.dma_start(
                        out=outT_d.ap()[o * 128 : (o + 1) * 128, t0 + hn : t0 + tn],
                        in_=ot[:, hn:],
                    )
                    return
                ot = o_pool.tile([128, tn], f32, name="ot", tag="ot")
                if o % 2 == 0:
                    nc.scalar.activation(
                        ot[:],
                        acc[:],
                        mybir.ActivationFunctionType.Identity,
                        bias=b_sb[:, o : o + 1],
                    )
                else:
                    nc.vector.tensor_scalar_add(ot[:], acc[:], b_sb[:, o : o + 1])
                nc.scalar.dma_start(
                    out=outT_d.ap()[o * 128 : (o + 1) * 128, t0 : t0 + tn],
                    in_=ot[:],
                )

            def load_x(t0, tn):
                # one 2 MB DMA per chunk: [128, ND, tn] tile, partition p
                # holds row 128d+p of xT for each d slice
                xt = x_pool.tile([128, ND, tn], f32r, name="xt", tag="xt")
                nc.sync.dma_start(
                    out=xt[:],
                    in_=xT_d.ap()[:, t0 : t0 + tn].rearrange(
                        "(s p) t -> p s t", p=128
                    ),
                )
                return xt

            def mm_groups(x_tiles, t0, tn, last=False):
                for o in range(NO):
                    acc = psum_pool.tile([128, tn], f32, name="acc", tag="acc")
                    for d in range(ND):
                        nc.tensor.matmul(
                            acc[:],
                            wts[o][d],
                            x_tiles[:, d],
                            start=(d == 0),
                            stop=(d == ND - 1),
                        )
                    evac_and_store(o, acc, t0, tn, parallel=last)

            # DMA issue order: b + all weights on the ACT ring first (4 MB,
            # fully landed by ~12us), x chunks on the SP ring — the first
            # matmul group waits ~15us for chunk-0 x, but after that the PE
            # is never weight-gated (loading weights after chunk-0 x instead
            # measured worse: it trades head position for pacing stalls).
            w_os = [load_w(o) for o in range(NO)]
            x_chunks = [load_x(0, TCH)]
            wts = [
                [w_os[o][:, d * 128 : (d + 1) * 128] for d in range(ND)]
                for o in range(NO)
            ]
            for t in range(NT):
                if t + 1 < NT:
                    x_chunks.append(load_x((t + 1) * TCH, TCH))
                mm_groups(x_chunks[t], t * TCH, TCH, last=(t == NT - 1))
    nc.compile()
    return nc


def _get_nc():
    global _nc
    if _nc is None:
        _nc = _build()
    return _nc


def _make_in_maps(x, W, b, V_r, C):
    Wt = (
        W.astype(np.float64).T + V_r.astype(np.float64) @ C.astype(np.float64).T
    ).astype(np.float32)
    # per-o SBUF image: Wb[o, p, 128d+c] = Wt[128d+p, 128o+c] so each
    # o-slice is one 512 KB DMA with 4 KB contiguous per partition
    Wb = np.ascontiguousarray(
        Wt.reshape(ND, 128, NO, 128).transpose(2, 1, 0, 3).reshape(NO, 128, ND * 128)
    )
    b = np.ascontiguousarray(b, dtype=np.float32)
    return [
        {
            "xT": np.ascontiguousarray(x[i].T.astype(np.float32, copy=False)),
            "Wb": Wb,
            "bb": b,
        }
        for i in range(N_CORES)
    ]


def _execute(in_maps, trace=False):
    from concourse.bass_utils import run_bass_kernel_spmd

    return run_bass_kernel_spmd(
        _get_nc(), in_maps, list(range(N_CORES)), trace=trace
    )


def kernel(x, W, b, V_r, C):
    res = _execute(_make_in_maps(x, W, b, V_r, C))
    out = np.empty((N_CORES, T, O), dtype=np.float32)
    for i in range(N_CORES):
        out[i] = res.results[i]["outT"].T
    return out


# revision 19
# speedup vs baseline: 1.0274x; 1.0110x over previous
"""CorrectedLinear on 8 TRN2 NeuronCores.

Math: out = x @ W.T + b + (x @ V_r) @ C.T
    = x @ (W.T + V_r @ C.T) + b          -- fold the rank-32 correction
      into a single effective weight matrix Wt [d_in, d_out] (0.05% of
      the GEMM FLOPs, done host-side in float64).

Sharding: pure data-parallel over the batch dim (8 batches -> 8 cores).
Each core computes a [8192, 1024] x [1024, 1024] GEMM.

Device layout: the PE contracts along the partition axis, so both
operands need d_in on partitions. x is fed pre-transposed per core
(xT [d_in, t]) and the output is produced transposed (outT [d_out, t]),
un-transposed on the host. All matmul operands use float32r (full-rate
fp32, ~1.5e-4 matmul precision) with fp32 PSUM accumulation.

Weights are fed as one 512 KB contiguous SBUF-image per 128-row output
slice (4 KB per partition per DMA) on the ACT HWDGE ring; x streams on
the SP ring in full 512-token chunks (N=512 keeps the 187 ns LDWEIGHTS
hidden under the 213 ns matmul stream; smaller chunks are LDW-bound).
"""

import numpy as np

N_CORES = 8
T = 8192          # tokens per core (batch entry)
D = 1024          # d_in
O = 1024          # d_out
TCH = 512         # moving free dim per matmul (= one PSUM bank of fp32)
NT = T // TCH     # 16 t-chunks
ND = D // 128     # 8 contraction slices
NO = O // 128     # 8 output-partition slices

_nc = None


def _build():
    import concourse.bacc as bacc
    import concourse.mybir as mybir
    import concourse.tile as tile

    f32 = mybir.dt.float32
    f32r = mybir.dt.float32r

    nc = bacc.Bacc(
        "TRN2", target_bir_lowering=False, debug=False, num_devices=N_CORES
    )
    xT_d = nc.dram_tensor("xT", [D, T], f32r, kind="ExternalInput")
    Wb_d = nc.dram_tensor("Wb", [NO, 128, ND * 128], f32r, kind="ExternalInput")
    b_d = nc.dram_tensor("bb", [O], f32, kind="ExternalInput")
    outT_d = nc.dram_tensor("outT", [O, T], f32, kind="ExternalOutput")

    with tile.TileContext(nc) as tc:
        with (
            tc.tile_pool(name="wt", bufs=1) as wt_pool,
            tc.tile_pool(name="bp", bufs=1) as b_pool,
            tc.tile_pool(name="xp", bufs=48) as x_pool,
            tc.tile_pool(name="op", bufs=8) as o_pool,
            tc.tile_pool(name="ps", bufs=8, space="PSUM") as psum_pool,
        ):
            b_sb = b_pool.tile([128, NO], f32, name="b_sb")
            nc.scalar.dma_start(
                out=b_sb[:], in_=b_d.ap().rearrange("(j p) -> p j", p=128)
            )

            def load_w(o):
                w = wt_pool.tile([128, ND * 128], f32r, name=f"w{o}", tag=f"w{o}")
                nc.scalar.dma_start(out=w[:], in_=Wb_d.ap()[o])
                return w

            def evac_and_store(o, acc, t0, tn, parallel=False):
                """PSUM -> SBUF (+bias) -> DRAM, alternating ACT/DVE.

                parallel=True splits the tile in half across both engines
                and both HWDGE rings to shorten the kernel tail."""
                if parallel:
                    hn = tn // 2
                    ot = o_pool.tile([128, tn], f32, name="ot", tag="ot")
                    nc.scalar.activation(
                        ot[:, :hn],
                        acc[:, :hn],
                        mybir.ActivationFunctionType.Identity,
                        bias=b_sb[:, o : o + 1],
                    )
                    nc.vector.tensor_scalar_add(
                        ot[:, hn:], acc[:, hn:], b_sb[:, o : o + 1]
                    )
                    nc.scalar.dma_start(
                        out=outT_d.ap()[o * 128 : (o + 1) * 128, t0 : t0 + hn],
                        in_=ot[:, :hn],
                    )
                    nc.sync.dma_start(
                        out=outT_d.ap()[o * 128 : (o + 1) * 128, t0 + hn : t0 + tn],
                        in_=ot[:, hn:],
                    )
                    return
                ot = o_pool.tile([128, tn], f32, name="ot", tag="ot")
                if o % 2 == 0:
                    nc.scalar.activation(
                        ot[:],
                        acc[:],
                        mybir.ActivationFunctionType.Identity,
                        bias=b_sb[:, o : o + 1],
                    )
                else:
                    nc.vector.tensor_scalar_add(ot[:], acc[:], b_sb[:, o : o + 1])
                nc.scalar.dma_start(
                    out=outT_d.ap()[o * 128 : (o + 1) * 128, t0 : t0 + tn],
                    in_=ot[:],
                )

            def load_x(t0, tn):
                xs = []
                for d in range(ND):
                    xt = x_pool.tile([128, tn], f32r, name="xt", tag="xt")
                    nc.sync.dma_start(
                        out=xt[:],
                        in_=xT_d.ap()[d * 128 : (d + 1) * 128, t0 : t0 + tn],
                    )
                    xs.append(xt)
                return xs

            def mm_groups(x_tiles, t0, tn, last=False):
                for o in range(NO):
                    acc = psum_pool.tile([128, tn], f32, name="acc", tag="acc")
                    for d in range(ND):
                        nc.tensor.matmul(
                            acc[:],
                            wts[o][d],
                            x_tiles[d][:],
                            start=(d == 0),
                            stop=(d == ND - 1),
                        )
                    evac_and_store(o, acc, t0, tn, parallel=last)

            # DMA issue order: b + all weights on the ACT ring first (4 MB,
            # fully landed by ~12us), x chunks on the SP ring — the first
            # matmul group waits ~15us for chunk-0 x, but after that the PE
            # is never weight-gated (loading weights after chunk-0 x instead
            # measured worse: it trades head position for pacing stalls).
            w_os = [load_w(o) for o in range(NO)]
            wts = [
                [w_os[o][:, d * 128 : (d + 1) * 128] for d in range(ND)]
                for o in range(NO)
            ]
            for t in range(NT):
                xs = load_x(t * TCH, TCH)
                mm_groups(xs, t * TCH, TCH)
    nc.compile()
    return nc


def _get_nc():
    global _nc
    if _nc is None:
        _nc = _build()
    return _nc


def _make_in_maps(x, W, b, V_r, C):
    Wt = (
        W.astype(np.float64).T + V_r.astype(np.float64) @ C.astype(np.float64).T
    ).astype(np.float32)
    # per-o SBUF image: Wb[o, p, 128d+c] = Wt[128d+p, 128o+c] so each
    # o-slice is one 512 KB DMA with 4 KB contiguous per partition
    Wb = np.ascontiguousarray(
        Wt.reshape(ND, 128, NO, 128).transpose(2, 1, 0, 3).reshape(NO, 128, ND * 128)
    )
    b = np.ascontiguousarray(b, dtype=np.float32)
    return [
        {
            "xT": np.ascontiguousarray(x[i].T.astype(np.float32, copy=False)),
            "Wb": Wb,
            "bb": b,
        }
        for i in range(N_CORES)
    ]


def _execute(in_maps, trace=False):
    from concourse.bass_utils import run_bass_kernel_spmd

    return run_bass_kernel_spmd(
        _get_nc(), in_maps, list(range(N_CORES)), trace=trace
    )


def kernel(x, W, b, V_r, C):
    res = _execute(_make_in_maps(x, W, b, V_r, C))
    out = np.empty((N_CORES, T, O), dtype=np.float32)
    for i in range(N_CORES):
        out[i] = res.results[i]["outT"].T
    return out
